# revision 1
# baseline (speedup 1.0000x reference)
"""MultiHeadAttention forward on 8 Trainium2 NeuronCores.

Tensor-parallel over heads: each core owns 2 of 16 heads (d_loc=256 of the
2048 QKV output columns, and the matching 256 rows of Wo). Each core
computes a full-shape partial output; the host sums the 8 partials and
adds bo.

Problem shape: x [2, 2048, 2048], 16 heads, d_k = 128, fp32.

Device-side layout choices (all matmuls fp32r, 1 cycle/row at N>=256):
  - x is fed pre-transposed (xT [C, B*T]) so projection contractions run
    over the partition dim with natural tiles.
  - Q, K are produced transposed (QT/KT [d, t]); V in natural [t, d].
    d_k = 128 = exactly one partition tile per head.
  - Scores are computed transposed: ST[tk, tq] = matmul(lhsT=KT-tile,
    rhs=QT-chunk); softmax needs no max-subtraction (|scores| ~ 5).
  - exp on ScalarE (PSUM->SBUF); denominator via matmul with an all-ones
    [128,128] stationary (broadcast row-sum over the partition dim);
    AV as matmul(lhsT=V-tile, rhs=expST) accumulating out^T [d, tq].
  - Normalization (1/denom) on DVE, applied to out^T chunks.
  - Output projection: lhsT = AVout^T slices, rhs = Wo rows.
"""

import functools
from contextlib import ExitStack

import numpy as np

D_MODEL = 2048
NUM_HEADS = 16
DK = 128
B = 2
T = 2048
BT = B * T
N_CORES = 8
H_LOC = NUM_HEADS // N_CORES  # 2 heads per core
D_LOC = H_LOC * DK  # 256
C_TILES = D_MODEL // 128  # 16
TQ = 512  # tq chunk width (one PSUM bank in fp32)
NCH = T // TQ  # 4 chunks per batch
TK_TILES = T // 128  # 16


def _body(ctx, tc, xT, wqkv, bqk, wo, y):
    import concourse.bass as bass  # noqa: F401
    from concourse import mybir

    nc = tc.nc
    f32 = mybir.dt.float32
    f32r = mybir.dt.float32r
    Exp = mybir.ActivationFunctionType.Exp
    inv_sqrt_dk = 1.0 / float(np.sqrt(DK))

    # ---------------- resident tensors ----------------
    # Interleave the first x-chunk's tile loads with the weight loads so the
    # first projection matmuls can start after ~2 DMAs instead of after the
    # whole 8.5 MB weight preload (was a 29 us PE gap at kernel start).
    wpool = ctx.enter_context(tc.tile_pool(name="wpool", bufs=1))
    x_pool = ctx.enter_context(tc.tile_pool(name="x_pool", bufs=20))

    w_tiles = []
    xt_pre = []
    for i in range(C_TILES):
        xti = x_pool.tile([128, TQ], f32r, tag="xt", name=f"xtpre{i}")
        nc.sync.dma_start(out=xti, in_=xT[i * 128 : (i + 1) * 128, 0:TQ])
        xt_pre.append(xti)
        wt = wpool.tile([128, 3 * D_LOC], f32r, tag=f"w{i}", name=f"w{i}")
        nc.sync.dma_start(out=wt, in_=xqkv_slice(wqkv, i))
        w_tiles.append(wt)
    bqk_sb = wpool.tile([128, 4], f32, tag="bqk", name="bqk")
    nc.sync.dma_start(out=bqk_sb, in_=bqk[:, :])

    wo_tiles = []
    for d in range(2):
        wot = wpool.tile([128, D_MODEL], f32r, tag=f"wo{d}", name=f"wo{d}")
        nc.sync.dma_start(out=wot, in_=wo[d * 128 : (d + 1) * 128, :])
        wo_tiles.append(wot)

    ones_f = wpool.tile([128, 128], f32, tag="ones_f", name="ones_f")
    nc.vector.memset(ones_f, 1.0)
    ones = wpool.tile([128, 128], f32r, tag="ones", name="ones")
    nc.vector.tensor_copy(ones, ones_f)

    # ---------------- pools ----------------
    qkv_pool = ctx.enter_context(tc.tile_pool(name="qkv_pool", bufs=1))
    av_pool = ctx.enter_context(tc.tile_pool(name="av_pool", bufs=1))
    es_pool = ctx.enter_context(tc.tile_pool(name="es_pool", bufs=5))
    rc_pool = ctx.enter_context(tc.tile_pool(name="rc_pool", bufs=1))
    y_pool = ctx.enter_context(tc.tile_pool(name="y_pool", bufs=2))

    ps_proj = ctx.enter_context(tc.tile_pool(name="ps_proj", bufs=2, space="PSUM"))
    ps_o = ctx.enter_context(tc.tile_pool(name="ps_o", bufs=2, space="PSUM"))
    ps_s = ctx.enter_context(tc.tile_pool(name="ps_s", bufs=2, space="PSUM"))
    ps_dn = ctx.enter_context(tc.tile_pool(name="ps_dn", bufs=1, space="PSUM"))
    ps_av = ctx.enter_context(tc.tile_pool(name="ps_av", bufs=1, space="PSUM"))

    for b in range(B):
        # ---------------- phase P: QKV projections ----------------
        qT = [
            qkv_pool.tile([128, T], f32r, tag=f"qT{d}", name=f"qT{d}_{b}")
            for d in range(2)
        ]
        kT = [
            qkv_pool.tile([128, T], f32r, tag=f"kT{d}", name=f"kT{d}_{b}")
            for d in range(2)
        ]
        # bufs=2: lets batch b+1's V projection write fresh slots while batch
        # b's attention is still reading the old ones (otherwise the WAR dep
        # stalls the whole b+1 projection behind the end of b's attention).
        v_t = [
            qkv_pool.tile([128, D_LOC], f32r, tag=f"v{t}", name=f"v{t}_{b}", bufs=2)
            for t in range(TK_TILES)
        ]

        for ch in range(NCH):
            t0 = b * T + ch * TQ
            if b == 0 and ch == 0:
                xt = xt_pre
            else:
                xt = []
                for i in range(C_TILES):
                    xti = x_pool.tile(
                        [128, TQ], f32r, tag="xt", name=f"xt{b}_{ch}_{i}"
                    )
                    nc.sync.dma_start(
                        out=xti, in_=xT[i * 128 : (i + 1) * 128, t0 : t0 + TQ]
                    )
                    xt.append(xti)

            # QT / KT: j -> (qT, kT)[j // 2][j % 2]
            for j, dest in enumerate((qT[0], qT[1], kT[0], kT[1])):
                ps = ps_proj.tile([128, TQ], f32, tag="proj", name=f"psqk{b}_{ch}_{j}")
                for i in range(C_TILES):
                    nc.tensor.matmul(
                        ps,
                        w_tiles[i][:, j * 128 : (j + 1) * 128],
                        xt[i],
                        start=(i == 0),
                        stop=(i == C_TILES - 1),
                    )
                # PSUM -> SBUF with per-partition bias add
                nc.vector.tensor_scalar_add(
                    dest[:, ch * TQ : (ch + 1) * TQ], ps, bqk_sb[:, j : j + 1]
                )

            # V: natural [t, d] layout; bias folded in as a K=1 matmul
            for ts in range(TQ // 128):
                t_idx = ch * (TQ // 128) + ts
                ps = ps_proj.tile([128, TQ], f32, tag="proj", name=f"psv{b}_{t_idx}")
                psv = ps[:, :D_LOC]
                for i in range(C_TILES):
                    nc.tensor.matmul(
                        psv,
                        xt[i][:, ts * 128 : (ts + 1) * 128],
                        w_tiles[i][:, 2 * D_LOC : 3 * D_LOC],
                        start=(i == 0),
                        stop=(i == C_TILES - 1),
                    )
                nc.vector.tensor_copy(v_t[t_idx], psv)

        # ---------------- phase A: attention per local head ----------------
        avT = [
            av_pool.tile([128, T], f32r, tag=f"avT{d}", name=f"avT{d}_{b}")
            for d in range(2)
        ]
        # b=0: head-outer so qT[0]/kT[0] release halfway through and batch 1's
        # projection can begin. b=1 (last batch): chunk-outer so both heads'
        # avT chunks complete together and phase O can start after chunk 0.
        def emit_o_chunk(ch):
            # output projection for the t-tiles whose avT chunk just finished
            for t in range(ch * (TQ // 128), (ch + 1) * (TQ // 128)):
                row0 = b * T + t * 128
                for half in range(2):
                    ystage = y_pool.tile(
                        [128, D_MODEL // 2],
                        f32,
                        tag="ystage",
                        name=f"ys{b}_{t}_{half}",
                    )
                    for q in range(2):
                        nch = half * 2 + q
                        ps = ps_o.tile(
                            [128, TQ], f32, tag="o", name=f"pso{b}_{t}_{nch}"
                        )
                        for d in range(2):
                            nc.tensor.matmul(
                                ps,
                                avT[d][:, t * 128 : (t + 1) * 128],
                                wo_tiles[d][:, nch * TQ : (nch + 1) * TQ],
                                start=(d == 0),
                                stop=(d == 1),
                            )
                        nc.vector.tensor_copy(ystage[:, q * TQ : (q + 1) * TQ], ps)
                    nc.sync.dma_start(
                        out=y[
                            row0 : row0 + 128,
                            half * (D_MODEL // 2) : (half + 1) * (D_MODEL // 2),
                        ],
                        in_=ystage,
                    )

        if b == 0:
            hc_order = [(h, ch) for h in range(H_LOC) for ch in range(NCH)]
        else:
            hc_order = [(h, ch) for ch in range(NCH) for h in range(H_LOC)]
        for h, ch in hc_order:
            if True:
                pav = ps_av.tile([128, TQ], f32, tag="av", name=f"pav{b}_{h}_{ch}")
                pdn = ps_dn.tile([128, TQ], f32, tag="dn", name=f"pdn{b}_{h}_{ch}")
                for tk in range(TK_TILES):
                    pss = ps_s.tile(
                        [128, TQ], f32, tag="s", name=f"pss{b}_{h}_{ch}_{tk}"
                    )
                    nc.tensor.matmul(
                        pss,
                        kT[h][:, tk * 128 : (tk + 1) * 128],
                        qT[h][:, ch * TQ : (ch + 1) * TQ],
                        start=True,
                        stop=True,
                    )
                    es = es_pool.tile(
                        [128, TQ], f32r, tag="es", name=f"es{b}_{h}_{ch}_{tk}"
                    )
                    nc.scalar.activation(es, pss, Exp, scale=inv_sqrt_dk)
                    nc.tensor.matmul(
                        pdn,
                        ones[:, 0:128],
                        es,
                        start=(tk == 0),
                        stop=(tk == TK_TILES - 1),
                    )
                    nc.tensor.matmul(
                        pav,
                        v_t[tk][:, h * 128 : (h + 1) * 128],
                        es,
                        start=(tk == 0),
                        stop=(tk == TK_TILES - 1),
                    )
                rc = rc_pool.tile([128, TQ], f32, tag="rc", name=f"rc{b}_{h}_{ch}")
                nc.vector.reciprocal_approx_fast(out=rc, in_=pdn)
                nc.vector.tensor_mul(
                    avT[h][:, ch * TQ : (ch + 1) * TQ], pav, rc
                )
            if b == B - 1 and h == H_LOC - 1:
                emit_o_chunk(ch)

        if b < B - 1:
            for ch in range(NCH):
                emit_o_chunk(ch)




def xqkv_slice(wqkv, i):
    return wqkv[i * 128 : (i + 1) * 128, :]


@functools.cache
def _build():
    from concourse import bacc
    import concourse.tile as tile
    from concourse import mybir

    nc = bacc.Bacc(
        "TRN2",
        target_bir_lowering=False,
        debug=False,
        enable_asserts=False,
        num_devices=N_CORES,
    )
    f32 = mybir.dt.float32
    f32r = mybir.dt.float32r
    xT = nc.dram_tensor("xT", [D_MODEL, BT], f32r, kind="ExternalInput").ap()
    wqkv = nc.dram_tensor(
        "wqkv", [D_MODEL, 3 * D_LOC], f32r, kind="ExternalInput"
    ).ap()
    bqk = nc.dram_tensor("bqk", [128, 4], f32, kind="ExternalInput").ap()
    wo = nc.dram_tensor("wo", [D_LOC, D_MODEL], f32r, kind="ExternalInput").ap()
    y = nc.dram_tensor("y", [BT, D_MODEL], f32, kind="ExternalOutput").ap()

    with tile.TileContext(nc) as tc:
        with ExitStack() as ctx:
            _body(ctx, tc, xT, wqkv, bqk, wo, y)
    nc.compile()
    return nc


def _shard_inputs(x, Wq, bq, Wk, bk, Wv, bv, Wo, bo):
    """Host-side sharding: returns per-core input maps."""
    f = np.float32
    xT = np.ascontiguousarray(np.asarray(x, f).reshape(BT, D_MODEL).T)
    Wq, Wk, Wv, Wo = (np.asarray(a, f) for a in (Wq, Wk, Wv, Wo))
    bq, bk, bv = (np.asarray(a, f) for a in (bq, bk, bv))
    in_maps = []
    for c in range(N_CORES):
        sl = slice(c * D_LOC, (c + 1) * D_LOC)
        wqkv_pad = np.ascontiguousarray(
            np.concatenate([Wq[:, sl], Wk[:, sl], Wv[:, sl]], axis=1)
        )
        bqk_t = np.ascontiguousarray(
            np.stack(
                [
                    bq[sl][:128],
                    bq[sl][128:],
                    bk[sl][:128],
                    bk[sl][128:],
                ],
                axis=1,
            )
        )
        wo_loc = np.ascontiguousarray(Wo[sl, :])
        in_maps.append({"xT": xT, "wqkv": wqkv_pad, "bqk": bqk_t, "wo": wo_loc})
    return in_maps


def _run(in_maps, trace=False, **kwargs):
    from concourse.bass_utils import run_bass_kernel_spmd

    nc = _build()
    return run_bass_kernel_spmd(
        nc, in_maps, core_ids=list(range(N_CORES)), trace=trace, **kwargs
    )


def kernel(x, Wq, bq, Wk, bk, Wv, bv, Wo, bo):
    in_maps = _shard_inputs(x, Wq, bq, Wk, bk, Wv, bv, Wo, bo)
    res = _run(in_maps, trace=False)
    acc = np.zeros((BT, D_MODEL), np.float32)
    for rmap in res.results:
        acc += rmap["y"]
    acc += np.asarray(bo, np.float32)[None, :]
    acc += (np.asarray(bv, np.float32) @ np.asarray(Wo, np.float32))[None, :]
    return acc.reshape(B, T, D_MODEL)



# revision 2
# speedup vs baseline: 1.1608x; 1.1608x over previous
"""MultiHeadAttention forward on 8 Trainium2 NeuronCores.

Tensor-parallel over heads: each core owns 2 of 16 heads (d_loc=256 of the
2048 QKV output columns, and the matching 256 rows of Wo). Each core
computes a full-shape partial output; the host sums the 8 partials and
adds bo (+ bv @ Wo for the folded V bias).

Problem shape: x [2, 2048, 2048], 16 heads, d_k = 128; device math in
bf16 (tolerance is 2e-2; bf16 keeps ~1e-2) with fp32 PSUM accumulation.

Why bf16: stream cost on the PE is 1 cycle/row for both fp32r and bf16,
but LDWEIGHTS time halves for 2-byte weights via the compiler-automatic
FWL fast path (fp32/fp32r can never use it), and every stationary here
reloads per matmul. bf16 also halves DMA bytes and enables the DVE
2x_1p element rate for the softmax-denominator accumulation.

Layout (as in the fp32r version):
  - x fed pre-transposed (xT [C, B*T]); Q, K produced transposed
    (QT/KT [d, t]); V natural [t, d]; scores transposed ST[tk, tq] =
    matmul(lhsT=KT-tile, rhs=QT-chunk); no max-subtraction (|s| ~ 5).
  - exp on ScalarE in [128, 1024] tiles (two 512-wide score banks per
    activation, halving per-instruction overhead).
  - softmax denominator: DVE accumulates the 16 exp tiles per (head,
    chunk) into one bf16 tile, then a single ones-matmul reduces over
    the partition dim (replaces 16 PE matmuls per chunk).
  - AV as matmul(lhsT=V-tile, rhs=expST) accumulating out^T [d, tq];
    1/denom on DVE applied to out^T chunks; output projection from
    avT slices against Wo rows; y written bf16, summed on host.

Emission order keeps the in-order PE fed while ScalarE paces the
attention inner loop: proj(b0) | attn(b0)+proj(b1) fills | proj(b1)
drain | attn(b1)+output-projection fills | tail.
"""

import functools
from contextlib import ExitStack

import numpy as np

D_MODEL = 2048
NUM_HEADS = 16
DK = 128
B = 2
T = 2048
BT = B * T
N_CORES = 8
H_LOC = NUM_HEADS // N_CORES  # 2 heads per core
D_LOC = H_LOC * DK  # 256
C_TILES = D_MODEL // 128  # 16
TQ = 512  # tq chunk width (one PSUM bank in fp32)
NCH = T // TQ  # 4 chunks per batch
TK_TILES = T // 128  # 16


def _body(ctx, tc, xT, wqkv, bqk, wo, y):
    import concourse.bass as bass  # noqa: F401
    from concourse import mybir

    nc = tc.nc
    f32 = mybir.dt.float32
    bf16 = mybir.dt.bfloat16
    Exp = mybir.ActivationFunctionType.Exp
    Add = mybir.AluOpType.add
    Bypass = mybir.AluOpType.bypass
    inv_sqrt_dk = 1.0 / float(np.sqrt(DK))

    # ---------------- resident tensors ----------------
    # Interleave the first x-chunk's tile loads with the weight loads so the
    # first projection matmuls can start after ~2 DMAs.
    wpool = ctx.enter_context(tc.tile_pool(name="wpool", bufs=1))
    x_pool = ctx.enter_context(tc.tile_pool(name="x_pool", bufs=20))

    w_tiles = []
    xt_pre = []
    for i in range(C_TILES):
        xti = x_pool.tile([128, TQ], bf16, tag="xt", name=f"xtpre{i}")
        nc.sync.dma_start(out=xti, in_=xT[i * 128 : (i + 1) * 128, 0:TQ])
        xt_pre.append(xti)
        wt = wpool.tile([128, 3 * D_LOC], bf16, tag=f"w{i}", name=f"w{i}")
        nc.sync.dma_start(out=wt, in_=wqkv[i * 128 : (i + 1) * 128, :])
        w_tiles.append(wt)
    bqk_sb = wpool.tile([128, 4], f32, tag="bqk", name="bqk")
    nc.sync.dma_start(out=bqk_sb, in_=bqk[:, :])

    wo_tiles = []
    for d in range(2):
        wot = wpool.tile([128, D_MODEL], bf16, tag=f"wo{d}", name=f"wo{d}")
        nc.sync.dma_start(out=wot, in_=wo[d * 128 : (d + 1) * 128, :])
        wo_tiles.append(wot)

    ones = wpool.tile([128, 128], bf16, tag="ones", name="ones")
    nc.vector.memset(ones, 1.0)

    # ---------------- pools ----------------
    qkv_pool = ctx.enter_context(tc.tile_pool(name="qkv_pool", bufs=1))
    av_pool = ctx.enter_context(tc.tile_pool(name="av_pool", bufs=1))
    es_pool = ctx.enter_context(tc.tile_pool(name="es_pool", bufs=4))
    acc_pool = ctx.enter_context(tc.tile_pool(name="acc_pool", bufs=2))
    rc_pool = ctx.enter_context(tc.tile_pool(name="rc_pool", bufs=2))
    y_pool = ctx.enter_context(tc.tile_pool(name="y_pool", bufs=3))

    # PSUM budget (8 banks): ps_po 2 (proj + outproj + denominator) +
    # ps_s 2x[128,1024] = 4 + ps_av 2.
    ps_po = ctx.enter_context(tc.tile_pool(name="ps_po", bufs=2, space="PSUM"))
    ps_s = ctx.enter_context(tc.tile_pool(name="ps_s", bufs=2, space="PSUM"))
    ps_av = ctx.enter_context(tc.tile_pool(name="ps_av", bufs=2, space="PSUM"))

    qT, kT, v_t, avT = {}, {}, {}, {}

    def alloc_batch(b):
        qT[b] = [
            qkv_pool.tile([128, T], bf16, tag=f"qT{d}", name=f"qT{d}_{b}", bufs=2)
            for d in range(2)
        ]
        kT[b] = [
            qkv_pool.tile([128, T], bf16, tag=f"kT{d}", name=f"kT{d}_{b}", bufs=2)
            for d in range(2)
        ]
        v_t[b] = [
            qkv_pool.tile(
                [128, D_LOC], bf16, tag=f"v{t}", name=f"v{t}_{b}", bufs=2
            )
            for t in range(TK_TILES)
        ]
        avT[b] = [
            av_pool.tile([128, T], bf16, tag=f"avT{d}", name=f"avT{d}_{b}", bufs=2)
            for d in range(2)
        ]

    def emit_xt_dma(b, ch):
        t0 = b * T + ch * TQ
        xt = []
        for i in range(C_TILES):
            xti = x_pool.tile([128, TQ], bf16, tag="xt", name=f"xt{b}_{ch}_{i}")
            nc.sync.dma_start(
                out=xti, in_=xT[i * 128 : (i + 1) * 128, t0 : t0 + TQ]
            )
            xt.append(xti)
        return xt

    def emit_qk_unit(b, ch, j, xt):
        # j -> (q0, q1, k0, k1)
        dest = (qT[b][0], qT[b][1], kT[b][0], kT[b][1])[j]
        ps = ps_po.tile([128, TQ], f32, tag="po", name=f"psqk{b}_{ch}_{j}")
        for i in range(C_TILES):
            nc.tensor.matmul(
                ps,
                w_tiles[i][:, j * 128 : (j + 1) * 128],
                xt[i],
                start=(i == 0),
                stop=(i == C_TILES - 1),
            )
        # PSUM -> SBUF with per-partition bias add
        nc.vector.tensor_scalar_add(
            dest[:, ch * TQ : (ch + 1) * TQ], ps, bqk_sb[:, j : j + 1]
        )

    def emit_v_unit(b, ch, ts, xt):
        t_idx = ch * (TQ // 128) + ts
        ps = ps_po.tile([128, TQ], f32, tag="po", name=f"psv{b}_{t_idx}")
        psv = ps[:, :D_LOC]
        for i in range(C_TILES):
            nc.tensor.matmul(
                psv,
                xt[i][:, ts * 128 : (ts + 1) * 128],
                w_tiles[i][:, 2 * D_LOC : 3 * D_LOC],
                start=(i == 0),
                stop=(i == C_TILES - 1),
            )
        nc.vector.tensor_copy(v_t[b][t_idx], psv)

    def emit_attn_unit(b, h, ch):
        pav = ps_av.tile([128, TQ], f32, tag="av", name=f"pav{b}_{h}_{ch}")
        acc = acc_pool.tile([128, TQ], bf16, tag="acc", name=f"acc{b}_{h}_{ch}")
        q_sl = qT[b][h][:, ch * TQ : (ch + 1) * TQ]
        for tp in range(TK_TILES // 2):
            pss = ps_s.tile(
                [128, 2 * TQ], f32, tag="s", name=f"pss{b}_{h}_{ch}_{tp}"
            )
            es = es_pool.tile(
                [128, 2 * TQ], bf16, tag="es", name=f"es{b}_{h}_{ch}_{tp}"
            )
            for half in range(2):
                tk = 2 * tp + half
                nc.tensor.matmul(
                    pss[:, half * TQ : (half + 1) * TQ],
                    kT[b][h][:, tk * 128 : (tk + 1) * 128],
                    q_sl,
                    start=True,
                    stop=True,
                )
            nc.scalar.activation(es, pss, Exp, scale=inv_sqrt_dk)
            for half in range(2):
                tk = 2 * tp + half
                nc.tensor.matmul(
                    pav,
                    v_t[b][tk][:, h * 128 : (h + 1) * 128],
                    es[:, half * TQ : (half + 1) * TQ],
                    start=(tk == 0),
                    stop=(tk == TK_TILES - 1),
                )
            with nc.allow_low_precision("softmax denominator partials, bf16"):
                if tp == 0:
                    nc.vector.scalar_tensor_tensor(
                        acc, es[:, :TQ], 0.0, es[:, TQ:], Bypass, Add
                    )
                else:
                    nc.vector.tensor_add(acc, acc, es[:, :TQ])
                    nc.vector.tensor_add(acc, acc, es[:, TQ:])
        # single partition-dim reduction of the accumulated exp sums
        pdn = ps_po.tile([128, TQ], f32, tag="po", name=f"pdn{b}_{h}_{ch}")
        nc.tensor.matmul(pdn, ones[:, 0:128], acc, start=True, stop=True)
        rc = rc_pool.tile([128, TQ], f32, tag="rc", name=f"rc{b}_{h}_{ch}")
        nc.vector.reciprocal_approx_fast(out=rc, in_=pdn)
        nc.vector.tensor_mul(avT[b][h][:, ch * TQ : (ch + 1) * TQ], pav, rc)

    def emit_o_chunk(b, ch):
        for t in range(ch * (TQ // 128), (ch + 1) * (TQ // 128)):
            row0 = b * T + t * 128
            for half in range(2):
                ystage = y_pool.tile(
                    [128, D_MODEL // 2],
                    bf16,
                    tag="ystage",
                    name=f"ys{b}_{t}_{half}",
                )
                for q in range(2):
                    nch_i = half * 2 + q
                    ps = ps_po.tile(
                        [128, TQ], f32, tag="po", name=f"pso{b}_{t}_{nch_i}"
                    )
                    for d in range(2):
                        nc.tensor.matmul(
                            ps,
                            avT[b][d][:, t * 128 : (t + 1) * 128],
                            wo_tiles[d][:, nch_i * TQ : (nch_i + 1) * TQ],
                            start=(d == 0),
                            stop=(d == 1),
                        )
                    nc.vector.tensor_copy(ystage[:, q * TQ : (q + 1) * TQ], ps)
                nc.sync.dma_start(
                    out=y[
                        row0 : row0 + 128,
                        half * (D_MODEL // 2) : (half + 1) * (D_MODEL // 2),
                    ],
                    in_=ystage,
                )

    # ---------------- S1: projections for batch 0 ----------------
    alloc_batch(0)
    for ch in range(NCH):
        xt = xt_pre if ch == 0 else emit_xt_dma(0, ch)
        for j in range(4):
            emit_qk_unit(0, ch, j, xt)
        for ts in range(4):
            emit_v_unit(0, ch, ts, xt)

    # ---------------- S2/S3: attn(b0) with proj(b1) interleave ----------
    alloc_batch(1)
    fills = []
    for ch in range(NCH):
        fills.append(("dma", 1, ch))
        for j in range(4):
            fills.append(("qk", 1, ch, j))
        for ts in range(4):
            fills.append(("v", 1, ch, ts))

    xt_cur = {}

    def run_fill(f):
        if f[0] == "dma":
            xt_cur[(f[1], f[2])] = emit_xt_dma(f[1], f[2])
        elif f[0] == "qk":
            emit_qk_unit(f[1], f[2], f[3], xt_cur[(f[1], f[2])])
        else:
            emit_v_unit(f[1], f[2], f[3], xt_cur[(f[1], f[2])])

    fi = 0
    for ch in range(NCH):
        for h in range(H_LOC):
            emit_attn_unit(0, h, ch)
            # one PE-sized fill per attention unit (DMA fills are free)
            if fi < len(fills):
                run_fill(fills[fi])
                fi += 1
                if fills[fi - 1][0] == "dma" and fi < len(fills):
                    run_fill(fills[fi])
                    fi += 1
    while fi < len(fills):
        run_fill(fills[fi])
        fi += 1

    # ---------------- S4: attn(b1) with output-projection interleave ----
    o_fills = [(0, 0), (0, 1), (0, 2), (0, 3), (1, 0), (1, 1), (1, 2)]
    oi = 0
    for ch in range(NCH):
        for h in range(H_LOC):
            emit_attn_unit(1, h, ch)
            if oi < len(o_fills):
                emit_o_chunk(*o_fills[oi])
                oi += 1
    emit_o_chunk(1, 3)


@functools.cache
def _build():
    from concourse import bacc
    import concourse.tile as tile
    from concourse import mybir

    nc = bacc.Bacc(
        "TRN2",
        target_bir_lowering=False,
        debug=False,
        enable_asserts=False,
        num_devices=N_CORES,
    )
    f32 = mybir.dt.float32
    bf16 = mybir.dt.bfloat16
    xT = nc.dram_tensor("xT", [D_MODEL, BT], bf16, kind="ExternalInput").ap()
    wqkv = nc.dram_tensor(
        "wqkv", [D_MODEL, 3 * D_LOC], bf16, kind="ExternalInput"
    ).ap()
    bqk = nc.dram_tensor("bqk", [128, 4], f32, kind="ExternalInput").ap()
    wo = nc.dram_tensor("wo", [D_LOC, D_MODEL], bf16, kind="ExternalInput").ap()
    y = nc.dram_tensor("y", [BT, D_MODEL], bf16, kind="ExternalOutput").ap()

    with tile.TileContext(nc) as tc:
        with ExitStack() as ctx:
            _body(ctx, tc, xT, wqkv, bqk, wo, y)
    nc.compile()
    return nc


def _shard_inputs(x, Wq, bq, Wk, bk, Wv, bv, Wo, bo):
    """Host-side sharding: returns per-core input maps."""
    import ml_dtypes

    bf = ml_dtypes.bfloat16
    f = np.float32
    xT = np.ascontiguousarray(
        np.asarray(x, f).reshape(BT, D_MODEL).T.astype(bf)
    )
    Wq, Wk, Wv, Wo = (np.asarray(a, f) for a in (Wq, Wk, Wv, Wo))
    bq, bk, bv = (np.asarray(a, f) for a in (bq, bk, bv))
    in_maps = []
    for c in range(N_CORES):
        sl = slice(c * D_LOC, (c + 1) * D_LOC)
        wqkv_pad = np.ascontiguousarray(
            np.concatenate([Wq[:, sl], Wk[:, sl], Wv[:, sl]], axis=1).astype(bf)
        )
        bqk_t = np.ascontiguousarray(
            np.stack(
                [
                    bq[sl][:128],
                    bq[sl][128:],
                    bk[sl][:128],
                    bk[sl][128:],
                ],
                axis=1,
            )
        )
        wo_loc = np.ascontiguousarray(Wo[sl, :].astype(bf))
        in_maps.append({"xT": xT, "wqkv": wqkv_pad, "bqk": bqk_t, "wo": wo_loc})
    return in_maps


def _run(in_maps, trace=False, **kwargs):
    from concourse.bass_utils import run_bass_kernel_spmd

    nc = _build()
    return run_bass_kernel_spmd(
        nc, in_maps, core_ids=list(range(N_CORES)), trace=trace, **kwargs
    )


def kernel(x, Wq, bq, Wk, bk, Wv, bv, Wo, bo):
    in_maps = _shard_inputs(x, Wq, bq, Wk, bk, Wv, bv, Wo, bo)
    res = _run(in_maps, trace=False)
    acc = np.zeros((BT, D_MODEL), np.float32)
    for rmap in res.results:
        acc += np.asarray(rmap["y"], dtype=np.float32)
    acc += np.asarray(bo, np.float32)[None, :]
    acc += (np.asarray(bv, np.float32) @ np.asarray(Wo, np.float32))[None, :]
    return acc.reshape(B, T, D_MODEL)
